# revision 26
# baseline (speedup 1.0000x reference)
"""MultiHeadAttention Trainium2 kernel (8 NeuronCores).

Problem: B=2, S=2048, E=1024, H=16, HD=64.
  qg = q @ Wq + bq ; qh[h] = qg @ Whq[h] + bhq[h]   (same for k, v)
  scores = qh @ kh^T / sqrt(HD), masked (-inf where mask), softmax
  out = concat_h(softmax @ vh) @ Wo + bo

Sharding: core c = 4*b + g handles batch b, heads 4g..4g+3 (data parallel on
B, tensor parallel on H). The global+per-head projections are folded on the
host into per-head fused weights Fq[h] = Wq @ Whq[h] (etc.); the output
projection is row-sharded (fp16 partials summed on the host).

Pipeline: the exp ACTIVATE (one [128,1024] per (kc, head-pair), ~1.11us) is
the steady-state clock; everything else is laid out to stay under it:
  - ALL input DMAs ride the sync HWDGE queue -- a DMA issued from the scalar
    engine blocks every later ACT instruction (the exps) until the
    descriptor drains, which is what used to stall the head for ~15us.
    Order: weights, kT, qT, WB2; vT chunks + per-body masks follow.
  - One global body stream: body g=(qc,kc) runs QK pair matmuls + exp +
    mask-mult; the AV matmuls for body g-DEFER run DEFER bodies later so
    the attention front never waits on vT DMA / V projection and PE
    insertions absorb into the AV slack.
  - K projection runs as 6 concurrent chains (psum banks: sT x2 + the
    not-yet-used out0-3), emitted e-major so the PE follows kT chunk
    arrival instead of stalling chain-by-chain.
  - V projection interleaves into qc0 bodies 6-15 and qc1 bodies 0-5;
    Q projection for qc+1 splits into four [128,256] quarter-chains at
    light bodies; output projection for the previous qc runs as eight
    single-eo half-chains at kc 11-15 / next-qc kc 0,1,5; normalize at
    kc 9,10; avs/recip at kc 6-8.
  - The keep-mask ships non-duplicated ([S,S] bf16, half the HBM traffic of
    the duplicated layout); one [128,512] tile multiplies both esc halves.
  - Scores keep the [k, q] orientation; a ones-column in the V projection
    makes the AV matmul also produce softmax denominators; psum: "sT" tag
    double-buffered (4 banks) hosts scores + all borrowed slots, out0-3
    host the AV accumulators.  Evictions are DVE high-priority (GPSIMD
    offload was tried and costs more in power-throttle than it saves;
    GPSIMD also cannot touch PSUM).
"""
import ml_dtypes
import numpy as np
from contextlib import ExitStack

import concourse.bass as bass
import concourse.mybir as mybir
import concourse.tile as tile
from concourse import bacc

dt = mybir.dt
AF = mybir.ActivationFunctionType
OP = mybir.AluOpType

B, S, E, H = 2, 2048, 1024, 16
HD = E // H          # 64
HPC = H // 4         # heads per core = 4
N_CORES = 8
ECH = E // 128       # 8 e-chunks
NQ = S // 512        # 4 q chunks
NK = S // 128        # 16 k chunks
DEFER = 7            # AV deferral depth in bodies

# weights tensor column layouts (bf16)
WK_COLS = 2048
WQ_COLS = 2048
# WB2: Fv (8x260) | Wo (2x1024) | bfv (260) | ones (128)
W2_FV = 0
W2_WO = W2_FV + 2080
W2_BFV = W2_WO + 2048
W2_ONES = W2_BFV + 260
W2_COLS = W2_ONES + 128   # 4516

_prog_cache = {}


def build_program():
    if "nc" in _prog_cache:
        return _prog_cache["nc"]
    nc = bacc.Bacc("TRN2", target_bir_lowering=False, debug=False,
                   num_devices=N_CORES)

    bf = dt.bfloat16
    WBkt = nc.dram_tensor("WBk", [128, WK_COLS], bf, kind="ExternalInput").ap()
    WBqt = nc.dram_tensor("WBq", [128, WQ_COLS], bf, kind="ExternalInput").ap()
    WB2t = nc.dram_tensor("WB2", [128, W2_COLS], bf, kind="ExternalInput").ap()
    WFt = nc.dram_tensor("WF", [128, 4], dt.float32, kind="ExternalInput").ap()
    selt = nc.dram_tensor("sel", [128, 256], dt.float32r, kind="ExternalInput").ap()
    qT = nc.dram_tensor("qT", [E, S], bf, kind="ExternalInput").ap()
    kT = nc.dram_tensor("kT", [E, S], bf, kind="ExternalInput").ap()
    vT = nc.dram_tensor("vT", [E, S], bf, kind="ExternalInput").ap()
    # keep-mask, transposed: [S(k), S(q)]; one [128,512] tile per (kc,qc)
    # is applied to both 512-halves of each pair's esc.
    maskT2 = nc.dram_tensor("maskT2", [S, S], bf, kind="ExternalInput").ap()
    out_pT = nc.dram_tensor("out_pT", [E, S], dt.float16, kind="ExternalOutput").ap()
    warm_out = nc.dram_tensor("warm_out", [128, 512], dt.float32, kind="ExternalOutput").ap()

    with tile.TileContext(nc) as tc:
        with ExitStack() as ctx:
            wc = ctx.enter_context(tc.tile_pool(name="wc", bufs=1))
            xin = ctx.enter_context(tc.tile_pool(name="xin", bufs=1))
            qk = ctx.enter_context(tc.tile_pool(name="qk", bufs=1))
            vhp = ctx.enter_context(tc.tile_pool(name="vhp", bufs=1))
            xTp = ctx.enter_context(tc.tile_pool(name="xTp", bufs=1))
            maskp = ctx.enter_context(tc.tile_pool(name="maskp", bufs=6))
            escp = ctx.enter_context(tc.tile_pool(name="escp", bufs=16))
            avnp = ctx.enter_context(tc.tile_pool(name="avnp", bufs=2))
            oev = ctx.enter_context(tc.tile_pool(name="oev", bufs=4))
            psa = ctx.enter_context(tc.tile_pool(name="psa", bufs=1, space="PSUM"))

            WBk = wc.tile([128, WK_COLS], bf, tag="WBk", name="WBk")
            WBq = wc.tile([128, WQ_COLS], bf, tag="WBq", name="WBq")
            WB2 = wc.tile([128, W2_COLS], bf, tag="WB2", name="WB2")
            WF = wc.tile([128, 4], dt.float32, tag="WF")
            sel_sb = wc.tile([128, 256], dt.float32r, tag="sel")

            def Fq_sb(e):
                return WBq[:, bass.ds(256 * e, 256)]

            def Fk_sb(e):
                return WBk[:, bass.ds(256 * e, 256)]

            def Fv_sb(e):
                return WB2[:, bass.ds(W2_FV + 260 * e, 260)]

            def Wo_sb(c):
                return WB2[:, bass.ds(W2_WO + 1024 * c, 1024)]

            bfv_sb = WB2[0:1, bass.ds(W2_BFV, 260)]
            onesb_sb = WB2[0:1, bass.ds(W2_ONES, 128)]
            bfq_sb = WF[:, 0:2]
            bfk_sb = WF[:, 2:4]

            qt = [xin.tile([128, S], bf, tag=f"qt{e}", name=f"qt{e}")
                  for e in range(ECH)]
            xt_k = [xin.tile([128, S], bf, tag=f"xk{e}", name=f"xk{e}")
                    for e in range(ECH)]
            xt_v = [xin.tile([128, S], bf, tag=f"xv{e}", name=f"xv{e}")
                    for e in range(ECH)]
            # ALL input DMAs ride the sync HWDGE queue: a DMA issued from
            # the scalar engine blocks every later ACT instruction (exps!)
            # until the descriptor drains.  Order: weights, kT, qT, WB2;
            # vT chunks + masks interleave into the early bodies.
            nc.sync.dma_start(WF[:], WFt)
            nc.sync.dma_start(sel_sb[:], selt)
            nc.sync.dma_start(WBk[:], WBkt)
            nc.scalar.dma_start(WBq[:], WBqt)
            for e in range(4):
                nc.sync.dma_start(xt_k[e][:], kT[bass.ts(e, 128), :])
            for e in range(4, ECH):
                nc.scalar.dma_start(xt_k[e][:], kT[bass.ts(e, 128), :])
            for e in range(4):
                nc.sync.dma_start(qt[e][:], qT[bass.ts(e, 128), :])
            for e in range(4, ECH):
                nc.scalar.dma_start(qt[e][:], qT[bass.ts(e, 128), :])
            nc.sync.dma_start(WB2[:], WB2t)

            qhT = [qk.tile([128, S], bf, tag=f"qhT{p}", name=f"qhT{p}") for p in range(2)]
            khT = [qk.tile([128, S], bf, tag=f"khT{p}", name=f"khT{p}") for p in range(2)]
            vh_sb = [vhp.tile([128, 4, 65], bf, tag=f"vh{sc}", name=f"vh{sc}", bufs=1)
                     for sc in range(NK)]
            for sc in range(NK):
                nc.vector.memset(vh_sb[sc][:, :, 64:65], 1.0)
            xT_sb = [xTp.tile([128, S], bf, tag=f"xT{c}", name=f"xT{c}") for c in range(2)]

            # ---- PE warm-up (HAM) on a memset tile: no DMA dependency ----
            wmt = oev.tile([128, 512], bf, tag="wmt", name="wmt", bufs=1)
            nc.vector.memset(wmt[:], 1.0)
            wps = psa.tile([128, 512], dt.float32, tag="sT", name="wps", bufs=2)
            for i in range(24):
                nc.tensor.matmul(wps[:], wmt[:, 0:128], wmt[:],
                                 start=(i == 0), stop=(i == 23))
            wsb = oev.tile([128, 512], dt.float32, tag="wsb", name="wsb", bufs=1)
            nc.vector.tensor_copy(wsb[:], wps[:])
            nc.sync.dma_start(warm_out, wsb[:])
            # dummy exp: pays the ~2.7us ACT table load during the head.
            tldw = oev.tile([128, 1], bf, tag="tldw", name="tldw", bufs=1)
            nc.scalar.activation(tldw[:], wmt[:, 0:1], AF.Exp)

            # ---- K projection: 6 chains run concurrently (banks: sT x2 +
            # the not-yet-used out0-3), emitted e-major so the PE follows
            # kT chunk arrival instead of stalling chain-by-chain. ----
            kchains = [(p, nn) for p in range(2) for nn in range(4)]
            pps = []
            for ci in range(6):
                if ci < 2:
                    pps.append(psa.tile([128, 512], dt.float32, tag="sT",
                                        name="pp", bufs=2))
                else:
                    pps.append(psa.tile([128, 512], dt.float32,
                                        tag=f"out{ci - 2}", name="pp",
                                        bufs=1))
            for e in range(ECH):
                for ci in range(6):
                    pair, nn = kchains[ci]
                    nc.tensor.matmul(
                        pps[ci][:],
                        Fk_sb(e)[:, bass.ts(pair, 128)],
                        xt_k[e][:, bass.ts(nn, 512)],
                        start=(e == 0), stop=(e == ECH - 1),
                    )
            for ci in range(6):
                pair, nn = kchains[ci]
                nc.vector.tensor_scalar(
                    khT[pair][:, bass.ts(nn, 512)], pps[ci][:],
                    bfk_sb[:, bass.ds(pair, 1)], None, op0=OP.add,
                )
            for ci in range(6, 8):
                pair, nn = kchains[ci]
                pp = psa.tile([128, 512], dt.float32, tag="sT",
                              name="pp", bufs=2)
                for e in range(ECH):
                    nc.tensor.matmul(
                        pp[:],
                        Fk_sb(e)[:, bass.ts(pair, 128)],
                        xt_k[e][:, bass.ts(nn, 512)],
                        start=(e == 0), stop=(e == ECH - 1),
                    )
                nc.vector.tensor_scalar(
                    khT[pair][:, bass.ts(nn, 512)], pp[:],
                    bfk_sb[:, bass.ds(pair, 1)], None, op0=OP.add,
                )

            def emit_qproj(qcq, pair):
                # full [128,512] chain; used only for qc0 in the head
                pq = psa.tile([128, 512], dt.float32, tag="sT",
                              name="pq", bufs=2)
                for e in range(ECH):
                    nc.tensor.matmul(
                        pq[:],
                        Fq_sb(e)[:, bass.ts(pair, 128)],
                        qt[e][:, bass.ts(qcq, 512)],
                        start=(e == 0), stop=(e == ECH - 1),
                    )
                with tc.high_priority():
                    nc.vector.tensor_scalar(
                        qhT[pair][:, bass.ts(qcq, 512)], pq[:],
                        bfq_sb[:, bass.ds(pair, 1)], None, op0=OP.add,
                    )

            def emit_qproj_q(qcq, pair, col):
                # quarter chain: [128,256] psum, half a qc block of one pair
                pq = psa.tile([128, 256], dt.float32, tag="sT",
                              name="pq", bufs=2)
                for e in range(ECH):
                    nc.tensor.matmul(
                        pq[:],
                        Fq_sb(e)[:, bass.ts(pair, 128)],
                        qt[e][:, bass.ds(512 * qcq + 256 * col, 256)],
                        start=(e == 0), stop=(e == ECH - 1),
                    )
                with tc.high_priority():
                    nc.vector.tensor_scalar(
                        qhT[pair][:, bass.ds(512 * qcq + 256 * col, 256)],
                        pq[:], bfq_sb[:, bass.ds(pair, 1)], None, op0=OP.add,
                    )

            for pair in range(2):
                emit_qproj(0, pair)

            def emit_vproj(sc):
                pv = psa.tile([128, 4, 65], dt.float32, tag="sT",
                              name="pv", bufs=2)
                for e in range(ECH):
                    nc.tensor.matmul(
                        pv[:], xt_v[e][:, bass.ts(sc, 128)], Fv_sb(e),
                        start=(e == 0), stop=(e == ECH - 1),
                    )
                with tc.high_priority():
                    nc.vector.tensor_copy(vh_sb[sc][:, :, 0:64],
                                          pv[:, :, 0:64])

            # ---- attention ----
            sums128 = avnp.tile([128, 512], dt.float32, tag="sums128",
                                name="sums128", bufs=1)
            nc.vector.memset(sums128[:], 1.0)
            recip128 = avnp.tile([128, 512], dt.float32r, tag="recip128",
                                 name="recip128", bufs=1)
            avs = [avnp.tile([64, 512], dt.float32, tag=f"av{h}",
                             name=f"av{h}", bufs=1) for h in range(HPC)]

            from concourse.dve_ops import (
                RECIP_APPROX_FAST_CONSTS,
                RECIPROCAL_APPROX_FAST,
            )

            def emit_avs(prev_outs, h, eng="v"):
                with tc.high_priority():
                    if eng == "s":
                        nc.scalar.copy(avs[h][:], prev_outs[h][0:64, :])
                        nc.scalar.copy(sums128[32 * h:32 * h + 1, :],
                                       prev_outs[h][64:65, :])
                    else:
                        nc.vector.tensor_copy(avs[h][:], prev_outs[h][0:64, :])
                        nc.vector.tensor_copy(sums128[32 * h:32 * h + 1, :],
                                              prev_outs[h][64:65, :])

            def emit_recip():
                c = RECIP_APPROX_FAST_CONSTS
                with tc.high_priority():
                    nc.vector._custom_dve(
                        RECIPROCAL_APPROX_FAST,
                        out=recip128[:, :], in0=sums128[:, :],
                        s0=c["s0"], s1=c["s1"], imm2=c["imm2"],
                    )

            def emit_head_norm(h, pqc):
                pair, lo = h // 2, (h % 2) * 64
                bc = psa.tile([64, 512], dt.float32, tag="sT", name="bc",
                              bufs=2)
                nc.tensor.matmul(bc[:], sel_sb[:, bass.ds(64 * h, 64)],
                                 recip128[:], start=True, stop=True)
                with tc.high_priority():
                    nc.vector.tensor_tensor(
                        xT_sb[pair][lo:lo + 64, bass.ts(pqc, 512)],
                        avs[h][0:64, :], bc[:], op=OP.mult)

            def emit_oproj(eo, pqc, evict="v"):
                po = psa.tile([128, 512], dt.float32, tag="sT",
                              name="po", bufs=2)
                for c in range(2):
                    nc.tensor.matmul(
                        po[:],
                        Wo_sb(c)[:, bass.ts(eo, 128)],
                        xT_sb[c][:, bass.ts(pqc, 512)],
                        start=(c == 0), stop=(c == 1),
                    )
                ot = oev.tile([128, 512], dt.float16, tag="ot", name="ot",
                              bufs=3)
                with tc.high_priority():
                    if evict == "v":
                        nc.vector.tensor_copy(ot[:], po[:])
                    else:
                        nc.scalar.copy(ot[:], po[:])
                nc.sync.dma_start(
                    out_pT[bass.ts(eo, 128), bass.ts(pqc, 512)], ot[:])

            esc_hist = {}   # global body -> (esc, outs-index info)
            outs_cur = None
            outs_prev = None
            av_state = {"next": 0}

            def emit_av(g):
                nonlocal outs_cur, outs_prev
                aqc, akc = divmod(g, NK)
                if akc == 0:
                    outs_prev = outs_cur
                    outs_cur = [psa.tile([65, 512], dt.float32,
                                         tag=f"out{h}", name=f"out{h}")
                                for h in range(HPC)]
                esc = esc_hist.pop(g)
                for pair in range(2):
                    for half in range(2):
                        h = 2 * pair + half
                        nc.tensor.matmul(
                            outs_cur[h][:],
                            vh_sb[akc][:, h, :],
                            esc[pair][:, bass.ts(half, 512)],
                            start=(akc == 0), stop=(akc == NK - 1),
                        )

            for g in range(NQ * NK):
                qc, kc = divmod(g, NK)
                mt2 = maskp.tile([128, 512], bf, tag="mask", name="mask")
                nc.sync.dma_start(
                    mt2[:],
                    maskT2[bass.ts(kc, 128), bass.ts(qc, 512)])
                if g < 4:
                    for e in (2 * g, 2 * g + 1):
                        nc.sync.dma_start(xt_v[e][:], vT[bass.ts(e, 128), :])
                escs = []
                for pair in range(2):
                    sT = psa.tile([128, 1024], dt.float32, tag="sT",
                                  name="sT", bufs=2)
                    for half in range(2):
                        lo = half * 64
                        nc.tensor.matmul(
                            sT[:, bass.ts(half, 512)],
                            khT[pair][lo:lo + 64, bass.ts(kc, 128)],
                            qhT[pair][lo:lo + 64, bass.ts(qc, 512)],
                            start=True, stop=True,
                        )
                    esc = escp.tile([128, 1024], bf, tag="esc", name="esc")
                    nc.scalar.activation(esc[:], sT[:], AF.Exp)
                    nc.vector.tensor_tensor(
                        esc[:], esc[:],
                        mt2[:, None, :].broadcast_to([128, 2, 512]),
                        op=OP.mult)
                    escs.append(esc)
                esc_hist[g] = escs

                # avs h2,h3 must precede AV(qc,0) which reuses out banks
                if kc == 7 and qc >= 1:
                    emit_avs(outs_cur, 2)
                    emit_avs(outs_cur, 3)

                # qc3 catches up one extra AV per body at kc8-14 so the
                # last AV lands inside the loop (tail ran on a cold PE).
                n_extra = max(0, min(g, 62) - 55) if g >= 56 else 0
                want = g - DEFER + n_extra
                if g == NQ * NK - 1:
                    want = g
                # de-prioritize AV matmuls ~1 body so the next body's QK
                # pair jumps the PE queue ahead of them (AVs have DEFER
                # bodies of slack; the exp chain has none).
                with tc.high_priority(offset=-30):
                    while av_state["next"] <= want:
                        emit_av(av_state["next"])
                        av_state["next"] += 1

                # extras, emitted at the END of the body
                if kc == 6 and qc >= 1:
                    # outs of qc-1 just completed (AV(qc-1,15) above)
                    emit_avs(outs_cur, 0)
                    emit_avs(outs_cur, 1)
                elif kc == 8 and qc >= 1:
                    emit_recip()
                elif kc in (9, 10) and qc >= 1:
                    emit_head_norm(2 * (kc - 9), qc - 1)
                    emit_head_norm(2 * (kc - 9) + 1, qc - 1)
                elif kc in (11, 12, 13, 14, 15) and qc >= 1:
                    emit_oproj(kc - 11, qc - 1)
                elif kc in (0, 1) and qc >= 2:
                    emit_oproj(5 + kc, qc - 2)
                elif kc == 5 and qc >= 2:
                    emit_oproj(7, qc - 2)
                if qc == 0 and kc >= 6:
                    emit_vproj(kc - 6)
                elif qc == 1 and kc <= 5:
                    emit_vproj(10 + kc)
                qpk = (6, 7, 8, 9) if qc == 1 else (2, 3, 4, 5)
                if kc in qpk and qc < NQ - 1:
                    qq = qpk.index(kc)
                    emit_qproj_q(qc + 1, qq // 2, qq % 2)
                if qc == 0 and kc <= 1:
                    # HAM keepalive: PE idles at the attention start (no AVs
                    # yet) and drops to the K=4/8 clock gate.
                    hps = psa.tile([128, 512], dt.float32, tag="sT",
                                   name="hps", bufs=2)
                    for i in range(3):
                        nc.tensor.matmul(hps[:], wmt[:, 0:128], wmt[:],
                                         start=True, stop=True)

            # ---- tail: all AVs already emitted via qc3 catch-up ----
            # keepalive so the normalize/oproj chain stays at full clock
            hpt = psa.tile([128, 512], dt.float32, tag="sT", name="hpt",
                           bufs=2)
            for i in range(4):
                nc.tensor.matmul(hpt[:], wmt[:, 0:128], wmt[:],
                                 start=True, stop=True)
            for eo in (5, 6, 7):
                emit_oproj(eo, NQ - 2)
            for h in range(HPC):
                emit_avs(outs_cur, h, eng="s" if h % 2 else "v")
            emit_recip()
            for h in range(HPC):
                emit_head_norm(h, NQ - 1)
            for eo in range(ECH):
                emit_oproj(eo, NQ - 1, evict="v" if eo % 2 == 0 else "s")

    nc.compile()
    _prog_cache["nc"] = nc
    return nc


def prep_inputs(q_matrix, k_matrix, v_matrix, mask, Wq, bq, Wk, bk, Wv, bv,
                Whq, bhq, Whk, bhk, Whv, bhv, Wo, bo):
    f32 = np.float32
    bf16 = ml_dtypes.bfloat16
    q_matrix = np.asarray(q_matrix, f32)
    k_matrix = np.asarray(k_matrix, f32)
    v_matrix = np.asarray(v_matrix, f32)
    mask = np.asarray(mask)
    sc = f32(1.0 / np.sqrt(HD))

    Wq, Wk, Wv = np.asarray(Wq, f32), np.asarray(Wk, f32), np.asarray(Wv, f32)
    Whq, Whk, Whv = np.asarray(Whq, f32), np.asarray(Whk, f32), np.asarray(Whv, f32)
    bq, bk, bv = np.asarray(bq, f32), np.asarray(bk, f32), np.asarray(bv, f32)
    bhq, bhk, bhv = np.asarray(bhq, f32), np.asarray(bhk, f32), np.asarray(bhv, f32)
    # Fx[h] = Wx @ Whx[h]: one BLAS call via tensordot -> [E(out), H, HD]
    FqH = (np.tensordot(Wq, Whq, axes=([1], [1])) * sc).astype(f32)
    FkH = np.tensordot(Wk, Whk, axes=([1], [1])).astype(f32)
    FvH = np.tensordot(Wv, Whv, axes=([1], [1])).astype(f32)
    bqH = ((np.einsum("e,hed->hd", bq, Whq) + bhq) * sc).astype(f32)
    bkH = (np.einsum("e,hed->hd", bk, Whk) + bhk).astype(f32)
    bvH = (np.einsum("e,hed->hd", bv, Whv) + bhv).astype(f32)
    WoM = np.asarray(Wo, f32)

    sel = np.zeros((128, 256), f32)
    for h in range(4):
        sel[32 * h, 64 * h:64 * (h + 1)] = 1.0
    in_maps = []
    for core in range(N_CORES):
        b, g = core // 4, core % 4
        hs = [4 * g + j for j in range(4)]
        Fq_c = np.ascontiguousarray(FqH[:, hs, :].reshape(E, 256))
        Fk_c = np.ascontiguousarray(FkH[:, hs, :].reshape(E, 256))
        Fv_c = np.zeros((E, 260), f32)
        bfv_c = np.zeros((260,), f32)
        for j, h in enumerate(hs):
            Fv_c[:, 65 * j:65 * j + 64] = FvH[:, h, :]
            bfv_c[65 * j:65 * j + 64] = bvH[h]
            bfv_c[65 * j + 64] = 1.0
        bfq_c = np.stack([np.concatenate([bqH[hs[2 * p]], bqH[hs[2 * p + 1]]])
                          for p in range(2)], axis=1)                # [128, 2]
        bfk_c = np.stack([np.concatenate([bkH[hs[2 * p]], bkH[hs[2 * p + 1]]])
                          for p in range(2)], axis=1)
        Wo_c = WoM[256 * g:256 * (g + 1), :]                         # [256, 1024]

        WBqm = Fq_c.reshape(ECH, 128, 256).transpose(1, 0, 2).reshape(128, 2048)
        WBkm = Fk_c.reshape(ECH, 128, 256).transpose(1, 0, 2).reshape(128, 2048)
        WB2m = np.zeros((128, W2_COLS), f32)
        WB2m[:, W2_FV:W2_FV + 2080] = Fv_c.reshape(ECH, 128, 260).transpose(
            1, 0, 2).reshape(128, 2080)
        WB2m[:, W2_WO:W2_WO + 2048] = Wo_c.reshape(2, 128, 1024).transpose(
            1, 0, 2).reshape(128, 2048)
        WB2m[0, W2_BFV:W2_BFV + 260] = bfv_c
        WB2m[0, W2_ONES:W2_ONES + 128] = 1.0

        WFm = np.concatenate([bfq_c, bfk_c], axis=1)                 # [128, 4]

        # keep-mask, transposed: [S(k), S(q)]
        mk2 = (~mask[b].T).astype(f32)

        in_maps.append(dict(
            qT=np.ascontiguousarray(q_matrix[b].T).astype(bf16),
            kT=np.ascontiguousarray(k_matrix[b].T).astype(bf16),
            vT=np.ascontiguousarray(v_matrix[b].T).astype(bf16),
            maskT2=np.ascontiguousarray(mk2).astype(bf16),
            WBk=np.ascontiguousarray(WBkm).astype(bf16),
            WBq=np.ascontiguousarray(WBqm).astype(bf16),
            WB2=WB2m.astype(bf16), WF=WFm, sel=sel,
        ))
    return in_maps


def unshard(results, bo):
    bo = np.asarray(bo, np.float32)
    out = np.empty((B, S, E), np.float32)
    for b in range(B):
        acc = results[4 * b]["out_pT"].astype(np.float32)
        for g in range(1, 4):
            acc = acc + results[4 * b + g]["out_pT"].astype(np.float32)
        out[b] = acc.T + bo
    return out


def kernel(**inputs):
    from concourse.bass_utils import run_bass_kernel_spmd
    nc = build_program()
    in_maps = prep_inputs(**inputs)
    res = run_bass_kernel_spmd(nc, in_maps, list(range(N_CORES)))
    return unshard(res.results, inputs["bo"])


# revision 27
# speedup vs baseline: 1.0194x; 1.0194x over previous
"""MultiHeadAttention Trainium2 kernel (8 NeuronCores).

Problem: B=2, S=2048, E=1024, H=16, HD=64.
  qg = q @ Wq + bq ; qh[h] = qg @ Whq[h] + bhq[h]   (same for k, v)
  scores = qh @ kh^T / sqrt(HD), masked (-inf where mask), softmax
  out = concat_h(softmax @ vh) @ Wo + bo

Sharding: core c = 4*b + g handles batch b, heads 4g..4g+3 (data parallel on
B, tensor parallel on H). The global+per-head projections are folded on the
host into per-head fused weights Fq[h] = Wq @ Whq[h] (etc.); the output
projection is row-sharded (fp16 partials summed on the host).

Pipeline: the exp ACTIVATE (one [128,1024] per (kc, head-pair), ~1.11us) is
the steady-state clock; everything else is laid out to stay under it:
  - ALL input DMAs ride the sync HWDGE queue -- a DMA issued from the scalar
    engine blocks every later ACT instruction (the exps) until the
    descriptor drains, which is what used to stall the head for ~15us.
    Order: weights, kT, qT, WB2; vT chunks + per-body masks follow.
  - One global body stream: body g=(qc,kc) runs QK pair matmuls + exp +
    mask-mult; the AV matmuls for body g-DEFER run DEFER bodies later so
    the attention front never waits on vT DMA / V projection and PE
    insertions absorb into the AV slack.
  - K projection runs as 6 concurrent chains (psum banks: sT x2 + the
    not-yet-used out0-3), emitted e-major so the PE follows kT chunk
    arrival instead of stalling chain-by-chain.
  - V projection interleaves into qc0 bodies 6-15 and qc1 bodies 0-5;
    Q projection for qc+1 splits into four [128,256] quarter-chains at
    light bodies; output projection for the previous qc runs as eight
    single-eo half-chains at kc 11-15 / next-qc kc 0,1,5; normalize at
    kc 9,10; avs/recip at kc 6-8.
  - The keep-mask ships non-duplicated ([S,S] bf16, half the HBM traffic of
    the duplicated layout); one [128,512] tile multiplies both esc halves.
  - Scores keep the [k, q] orientation; a ones-column in the V projection
    makes the AV matmul also produce softmax denominators; psum: "sT" tag
    double-buffered (4 banks) hosts scores + all borrowed slots, out0-3
    host the AV accumulators.  Evictions are DVE high-priority (GPSIMD
    offload was tried and costs more in power-throttle than it saves;
    GPSIMD also cannot touch PSUM).
"""
import ml_dtypes
import numpy as np
from contextlib import ExitStack

import concourse.bass as bass
import concourse.mybir as mybir
import concourse.tile as tile
from concourse import bacc

dt = mybir.dt
AF = mybir.ActivationFunctionType
OP = mybir.AluOpType

B, S, E, H = 2, 2048, 1024, 16
HD = E // H          # 64
HPC = H // 4         # heads per core = 4
N_CORES = 8
ECH = E // 128       # 8 e-chunks
NQ = S // 512        # 4 q chunks
NK = S // 128        # 16 k chunks
DEFER = 7            # AV deferral depth in bodies

# weights tensor column layouts (bf16)
WK_COLS = 2048
WQ_COLS = 2048
# WB2: Fv (8x260) | Wo (2x1024) | bfv (260) | ones (128)
W2_FV = 0
W2_WO = W2_FV + 2080
W2_BFV = W2_WO + 2048
W2_ONES = W2_BFV + 260
W2_COLS = W2_ONES + 128   # 4516

_prog_cache = {}


def build_program():
    if "nc" in _prog_cache:
        return _prog_cache["nc"]
    nc = bacc.Bacc("TRN2", target_bir_lowering=False, debug=False,
                   num_devices=N_CORES)

    bf = dt.bfloat16
    WBkt = nc.dram_tensor("WBk", [128, WK_COLS], bf, kind="ExternalInput").ap()
    WBqt = nc.dram_tensor("WBq", [128, WQ_COLS], bf, kind="ExternalInput").ap()
    WB2t = nc.dram_tensor("WB2", [128, W2_COLS], bf, kind="ExternalInput").ap()
    WFt = nc.dram_tensor("WF", [128, 4], dt.float32, kind="ExternalInput").ap()
    selt = nc.dram_tensor("sel", [128, 256], dt.float32r, kind="ExternalInput").ap()
    qT = nc.dram_tensor("qT", [E, S], bf, kind="ExternalInput").ap()
    kT = nc.dram_tensor("kT", [E, S], bf, kind="ExternalInput").ap()
    vT = nc.dram_tensor("vT", [E, S], bf, kind="ExternalInput").ap()
    # keep-mask, transposed: [S(k), S(q)]; one [128,512] tile per (kc,qc)
    # is applied to both 512-halves of each pair's esc.
    maskT2 = nc.dram_tensor("maskT2", [S, S], bf, kind="ExternalInput").ap()
    out_pT = nc.dram_tensor("out_pT", [E, S], dt.float16, kind="ExternalOutput").ap()
    warm_out = nc.dram_tensor("warm_out", [128, 512], dt.float32, kind="ExternalOutput").ap()

    with tile.TileContext(nc) as tc:
        with ExitStack() as ctx:
            wc = ctx.enter_context(tc.tile_pool(name="wc", bufs=1))
            xin = ctx.enter_context(tc.tile_pool(name="xin", bufs=1))
            qk = ctx.enter_context(tc.tile_pool(name="qk", bufs=1))
            vhp = ctx.enter_context(tc.tile_pool(name="vhp", bufs=1))
            xTp = ctx.enter_context(tc.tile_pool(name="xTp", bufs=1))
            maskp = ctx.enter_context(tc.tile_pool(name="maskp", bufs=6))
            escp = ctx.enter_context(tc.tile_pool(name="escp", bufs=16))
            avnp = ctx.enter_context(tc.tile_pool(name="avnp", bufs=2))
            oev = ctx.enter_context(tc.tile_pool(name="oev", bufs=4))
            psa = ctx.enter_context(tc.tile_pool(name="psa", bufs=1, space="PSUM"))

            WBk = wc.tile([128, WK_COLS], bf, tag="WBk", name="WBk")
            WBq = wc.tile([128, WQ_COLS], bf, tag="WBq", name="WBq")
            WB2 = wc.tile([128, W2_COLS], bf, tag="WB2", name="WB2")
            WF = wc.tile([128, 4], dt.float32, tag="WF")
            sel_sb = wc.tile([128, 256], dt.float32r, tag="sel")

            def Fq_sb(e):
                return WBq[:, bass.ds(256 * e, 256)]

            def Fk_sb(e):
                return WBk[:, bass.ds(256 * e, 256)]

            def Fv_sb(e):
                return WB2[:, bass.ds(W2_FV + 260 * e, 260)]

            def Wo_sb(c):
                return WB2[:, bass.ds(W2_WO + 1024 * c, 1024)]

            bfv_sb = WB2[0:1, bass.ds(W2_BFV, 260)]
            onesb_sb = WB2[0:1, bass.ds(W2_ONES, 128)]
            bfq_sb = WF[:, 0:2]
            bfk_sb = WF[:, 2:4]

            qt = [xin.tile([128, S], bf, tag=f"qt{e}", name=f"qt{e}")
                  for e in range(ECH)]
            xt_k = [xin.tile([128, S], bf, tag=f"xk{e}", name=f"xk{e}")
                    for e in range(ECH)]
            xt_v = [xin.tile([128, S], bf, tag=f"xv{e}", name=f"xv{e}")
                    for e in range(ECH)]
            # ALL input DMAs ride the sync HWDGE queue: a DMA issued from
            # the scalar engine blocks every later ACT instruction (exps!)
            # until the descriptor drains.  Order: weights, kT, qT, WB2;
            # vT chunks + masks interleave into the early bodies.
            nc.sync.dma_start(WF[:], WFt)
            nc.sync.dma_start(sel_sb[:], selt)
            nc.sync.dma_start(WBk[:], WBkt)
            nc.scalar.dma_start(WBq[:], WBqt)
            for e in range(4):
                nc.sync.dma_start(xt_k[e][:], kT[bass.ts(e, 128), :])
            for e in range(4, ECH):
                nc.scalar.dma_start(xt_k[e][:], kT[bass.ts(e, 128), :])
            for e in range(ECH):
                nc.sync.dma_start(qt[e][:], qT[bass.ts(e, 128), :])
            nc.sync.dma_start(WB2[:], WB2t)

            qhT = [qk.tile([128, S], bf, tag=f"qhT{p}", name=f"qhT{p}") for p in range(2)]
            khT = [qk.tile([128, S], bf, tag=f"khT{p}", name=f"khT{p}") for p in range(2)]
            vh_sb = [vhp.tile([128, 4, 65], bf, tag=f"vh{sc}", name=f"vh{sc}", bufs=1)
                     for sc in range(NK)]
            for sc in range(NK):
                nc.vector.memset(vh_sb[sc][:, :, 64:65], 1.0)
            xT_sb = [xTp.tile([128, S], bf, tag=f"xT{c}", name=f"xT{c}") for c in range(2)]

            # ---- PE warm-up (HAM) on a memset tile: no DMA dependency ----
            wmt = oev.tile([128, 512], bf, tag="wmt", name="wmt", bufs=1)
            nc.vector.memset(wmt[:], 1.0)
            wps = psa.tile([128, 512], dt.float32, tag="sT", name="wps", bufs=2)
            for i in range(28):
                nc.tensor.matmul(wps[:], wmt[:, 0:128], wmt[:],
                                 start=(i == 0), stop=(i == 27))
            wsb = oev.tile([128, 512], dt.float32, tag="wsb", name="wsb", bufs=1)
            nc.vector.tensor_copy(wsb[:], wps[:])
            nc.sync.dma_start(warm_out, wsb[:])
            # dummy exp: pays the ~2.7us ACT table load during the head.
            tldw = oev.tile([128, 1], bf, tag="tldw", name="tldw", bufs=1)
            nc.scalar.activation(tldw[:], wmt[:, 0:1], AF.Exp)

            # ---- K projection: 6 chains run concurrently (banks: sT x2 +
            # the not-yet-used out0-3), emitted e-major so the PE follows
            # kT chunk arrival instead of stalling chain-by-chain. ----
            kchains = [(p, nn) for p in range(2) for nn in range(4)]
            pps = []
            for ci in range(6):
                if ci < 2:
                    pps.append(psa.tile([128, 512], dt.float32, tag="sT",
                                        name="pp", bufs=2))
                else:
                    pps.append(psa.tile([128, 512], dt.float32,
                                        tag=f"out{ci - 2}", name="pp",
                                        bufs=1))
            for e in range(ECH):
                for ci in range(6):
                    pair, nn = kchains[ci]
                    nc.tensor.matmul(
                        pps[ci][:],
                        Fk_sb(e)[:, bass.ts(pair, 128)],
                        xt_k[e][:, bass.ts(nn, 512)],
                        start=(e == 0), stop=(e == ECH - 1),
                    )
            for ci in range(6):
                pair, nn = kchains[ci]
                nc.vector.tensor_scalar(
                    khT[pair][:, bass.ts(nn, 512)], pps[ci][:],
                    bfk_sb[:, bass.ds(pair, 1)], None, op0=OP.add,
                )
            for ci in range(6, 8):
                pair, nn = kchains[ci]
                pp = psa.tile([128, 512], dt.float32, tag="sT",
                              name="pp", bufs=2)
                for e in range(ECH):
                    nc.tensor.matmul(
                        pp[:],
                        Fk_sb(e)[:, bass.ts(pair, 128)],
                        xt_k[e][:, bass.ts(nn, 512)],
                        start=(e == 0), stop=(e == ECH - 1),
                    )
                nc.vector.tensor_scalar(
                    khT[pair][:, bass.ts(nn, 512)], pp[:],
                    bfk_sb[:, bass.ds(pair, 1)], None, op0=OP.add,
                )

            def emit_qproj(qcq, pair):
                # full [128,512] chain; used only for qc0 in the head
                pq = psa.tile([128, 512], dt.float32, tag="sT",
                              name="pq", bufs=2)
                for e in range(ECH):
                    nc.tensor.matmul(
                        pq[:],
                        Fq_sb(e)[:, bass.ts(pair, 128)],
                        qt[e][:, bass.ts(qcq, 512)],
                        start=(e == 0), stop=(e == ECH - 1),
                    )
                with tc.high_priority():
                    nc.vector.tensor_scalar(
                        qhT[pair][:, bass.ts(qcq, 512)], pq[:],
                        bfq_sb[:, bass.ds(pair, 1)], None, op0=OP.add,
                    )

            def emit_qproj_q(qcq, pair, col):
                # quarter chain: [128,256] psum, half a qc block of one pair
                pq = psa.tile([128, 256], dt.float32, tag="sT",
                              name="pq", bufs=2)
                for e in range(ECH):
                    nc.tensor.matmul(
                        pq[:],
                        Fq_sb(e)[:, bass.ts(pair, 128)],
                        qt[e][:, bass.ds(512 * qcq + 256 * col, 256)],
                        start=(e == 0), stop=(e == ECH - 1),
                    )
                with tc.high_priority():
                    nc.vector.tensor_scalar(
                        qhT[pair][:, bass.ds(512 * qcq + 256 * col, 256)],
                        pq[:], bfq_sb[:, bass.ds(pair, 1)], None, op0=OP.add,
                    )

            for pair in range(2):
                emit_qproj(0, pair)

            def emit_vproj(sc):
                pv = psa.tile([128, 4, 65], dt.float32, tag="sT",
                              name="pv", bufs=2)
                for e in range(ECH):
                    nc.tensor.matmul(
                        pv[:], xt_v[e][:, bass.ts(sc, 128)], Fv_sb(e),
                        start=(e == 0), stop=(e == ECH - 1),
                    )
                with tc.high_priority():
                    nc.vector.tensor_copy(vh_sb[sc][:, :, 0:64],
                                          pv[:, :, 0:64])

            # ---- attention ----
            sums128 = avnp.tile([128, 512], dt.float32, tag="sums128",
                                name="sums128", bufs=1)
            nc.vector.memset(sums128[:], 1.0)
            recip128 = avnp.tile([128, 512], dt.float32r, tag="recip128",
                                 name="recip128", bufs=1)
            avs = [avnp.tile([64, 512], dt.float32, tag=f"av{h}",
                             name=f"av{h}", bufs=1) for h in range(HPC)]

            from concourse.dve_ops import (
                RECIP_APPROX_FAST_CONSTS,
                RECIPROCAL_APPROX_FAST,
            )

            def emit_avs(prev_outs, h, eng="v"):
                with tc.high_priority():
                    if eng == "s":
                        nc.scalar.copy(avs[h][:], prev_outs[h][0:64, :])
                        nc.scalar.copy(sums128[32 * h:32 * h + 1, :],
                                       prev_outs[h][64:65, :])
                    else:
                        nc.vector.tensor_copy(avs[h][:], prev_outs[h][0:64, :])
                        nc.vector.tensor_copy(sums128[32 * h:32 * h + 1, :],
                                              prev_outs[h][64:65, :])

            def emit_recip():
                c = RECIP_APPROX_FAST_CONSTS
                with tc.high_priority():
                    nc.vector._custom_dve(
                        RECIPROCAL_APPROX_FAST,
                        out=recip128[:, :], in0=sums128[:, :],
                        s0=c["s0"], s1=c["s1"], imm2=c["imm2"],
                    )

            def emit_head_norm(h, pqc):
                pair, lo = h // 2, (h % 2) * 64
                bc = psa.tile([64, 512], dt.float32, tag="sT", name="bc",
                              bufs=2)
                nc.tensor.matmul(bc[:], sel_sb[:, bass.ds(64 * h, 64)],
                                 recip128[:], start=True, stop=True)
                with tc.high_priority():
                    nc.vector.tensor_tensor(
                        xT_sb[pair][lo:lo + 64, bass.ts(pqc, 512)],
                        avs[h][0:64, :], bc[:], op=OP.mult)

            def emit_oproj(eo, pqc, evict="v"):
                po = psa.tile([128, 512], dt.float32, tag="sT",
                              name="po", bufs=2)
                for c in range(2):
                    nc.tensor.matmul(
                        po[:],
                        Wo_sb(c)[:, bass.ts(eo, 128)],
                        xT_sb[c][:, bass.ts(pqc, 512)],
                        start=(c == 0), stop=(c == 1),
                    )
                ot = oev.tile([128, 512], dt.float16, tag="ot", name="ot",
                              bufs=3)
                with tc.high_priority():
                    if evict == "v":
                        nc.vector.tensor_copy(ot[:], po[:])
                    else:
                        nc.scalar.copy(ot[:], po[:])
                nc.sync.dma_start(
                    out_pT[bass.ts(eo, 128), bass.ts(pqc, 512)], ot[:])

            esc_hist = {}   # global body -> (esc, outs-index info)
            outs_cur = None
            outs_prev = None
            av_state = {"next": 0}

            def emit_av(g):
                nonlocal outs_cur, outs_prev
                aqc, akc = divmod(g, NK)
                if akc == 0:
                    outs_prev = outs_cur
                    outs_cur = [psa.tile([65, 512], dt.float32,
                                         tag=f"out{h}", name=f"out{h}")
                                for h in range(HPC)]
                esc = esc_hist.pop(g)
                for pair in range(2):
                    for half in range(2):
                        h = 2 * pair + half
                        nc.tensor.matmul(
                            outs_cur[h][:],
                            vh_sb[akc][:, h, :],
                            esc[pair][:, bass.ts(half, 512)],
                            start=(akc == 0), stop=(akc == NK - 1),
                        )

            for g in range(NQ * NK):
                qc, kc = divmod(g, NK)
                mt2 = maskp.tile([128, 512], bf, tag="mask", name="mask")
                nc.sync.dma_start(
                    mt2[:],
                    maskT2[bass.ts(kc, 128), bass.ts(qc, 512)])
                if g < 4:
                    for e in (2 * g, 2 * g + 1):
                        nc.sync.dma_start(xt_v[e][:], vT[bass.ts(e, 128), :])
                escs = []
                for pair in range(2):
                    sT = psa.tile([128, 1024], dt.float32, tag="sT",
                                  name="sT", bufs=2)
                    for half in range(2):
                        lo = half * 64
                        nc.tensor.matmul(
                            sT[:, bass.ts(half, 512)],
                            khT[pair][lo:lo + 64, bass.ts(kc, 128)],
                            qhT[pair][lo:lo + 64, bass.ts(qc, 512)],
                            start=True, stop=True,
                        )
                    esc = escp.tile([128, 1024], bf, tag="esc", name="esc")
                    nc.scalar.activation(esc[:], sT[:], AF.Exp)
                    nc.vector.tensor_tensor(
                        esc[:], esc[:],
                        mt2[:, None, :].broadcast_to([128, 2, 512]),
                        op=OP.mult)
                    escs.append(esc)
                esc_hist[g] = escs

                # avs h2,h3 must precede AV(qc,0) which reuses out banks
                if kc == 7 and qc >= 1:
                    emit_avs(outs_cur, 2)
                    emit_avs(outs_cur, 3)

                # qc3 catches up one extra AV per body at kc8-14 so the
                # last AV lands inside the loop (tail ran on a cold PE).
                n_extra = max(0, min(g, 62) - 55) if g >= 56 else 0
                want = g - DEFER + n_extra
                if g == NQ * NK - 1:
                    want = g
                # de-prioritize AV matmuls ~1 body so the next body's QK
                # pair jumps the PE queue ahead of them (AVs have DEFER
                # bodies of slack; the exp chain has none).
                with tc.high_priority(offset=-30):
                    while av_state["next"] <= want:
                        emit_av(av_state["next"])
                        av_state["next"] += 1

                # extras, emitted at the END of the body
                if kc == 6 and qc >= 1:
                    # outs of qc-1 just completed (AV(qc-1,15) above)
                    emit_avs(outs_cur, 0)
                    emit_avs(outs_cur, 1)
                elif kc == 8 and qc >= 1:
                    emit_recip()
                elif kc in (9, 10) and qc >= 1:
                    emit_head_norm(2 * (kc - 9), qc - 1)
                    emit_head_norm(2 * (kc - 9) + 1, qc - 1)
                elif kc in (11, 12, 13, 14, 15) and qc >= 1:
                    emit_oproj(kc - 11, qc - 1)
                elif kc in (0, 1) and qc >= 2:
                    emit_oproj(5 + kc, qc - 2)
                elif kc == 5 and qc >= 2:
                    emit_oproj(7, qc - 2)
                if qc == 0 and kc >= 6:
                    emit_vproj(kc - 6)
                elif qc == 1 and kc <= 5:
                    emit_vproj(10 + kc)
                qpk = (6, 7, 8, 9) if qc == 1 else (2, 3, 4, 5)
                if kc in qpk and qc < NQ - 1:
                    qq = qpk.index(kc)
                    emit_qproj_q(qc + 1, qq // 2, qq % 2)
                if qc == 0 and kc <= 1:
                    # HAM keepalive: PE idles at the attention start (no AVs
                    # yet) and drops to the K=4/8 clock gate.
                    hps = psa.tile([128, 512], dt.float32, tag="sT",
                                   name="hps", bufs=2)
                    for i in range(3):
                        nc.tensor.matmul(hps[:], wmt[:, 0:128], wmt[:],
                                         start=True, stop=True)

            # ---- tail: all AVs already emitted via qc3 catch-up ----
            # keepalive so the normalize/oproj chain stays at full clock
            hpt = psa.tile([128, 512], dt.float32, tag="sT", name="hpt",
                           bufs=2)
            for i in range(4):
                nc.tensor.matmul(hpt[:], wmt[:, 0:128], wmt[:],
                                 start=True, stop=True)
            for eo in (5, 6, 7):
                emit_oproj(eo, NQ - 2)
            for h in range(HPC):
                emit_avs(outs_cur, h, eng="s" if h % 2 else "v")
            emit_recip()
            for h in range(HPC):
                emit_head_norm(h, NQ - 1)
            for eo in range(ECH):
                emit_oproj(eo, NQ - 1, evict="v" if eo % 2 == 0 else "s")

    nc.compile()
    _prog_cache["nc"] = nc
    return nc


def prep_inputs(q_matrix, k_matrix, v_matrix, mask, Wq, bq, Wk, bk, Wv, bv,
                Whq, bhq, Whk, bhk, Whv, bhv, Wo, bo):
    f32 = np.float32
    bf16 = ml_dtypes.bfloat16
    q_matrix = np.asarray(q_matrix, f32)
    k_matrix = np.asarray(k_matrix, f32)
    v_matrix = np.asarray(v_matrix, f32)
    mask = np.asarray(mask)
    sc = f32(1.0 / np.sqrt(HD))

    Wq, Wk, Wv = np.asarray(Wq, f32), np.asarray(Wk, f32), np.asarray(Wv, f32)
    Whq, Whk, Whv = np.asarray(Whq, f32), np.asarray(Whk, f32), np.asarray(Whv, f32)
    bq, bk, bv = np.asarray(bq, f32), np.asarray(bk, f32), np.asarray(bv, f32)
    bhq, bhk, bhv = np.asarray(bhq, f32), np.asarray(bhk, f32), np.asarray(bhv, f32)
    # Fx[h] = Wx @ Whx[h]: one BLAS call via tensordot -> [E(out), H, HD]
    FqH = (np.tensordot(Wq, Whq, axes=([1], [1])) * sc).astype(f32)
    FkH = np.tensordot(Wk, Whk, axes=([1], [1])).astype(f32)
    FvH = np.tensordot(Wv, Whv, axes=([1], [1])).astype(f32)
    bqH = ((np.einsum("e,hed->hd", bq, Whq) + bhq) * sc).astype(f32)
    bkH = (np.einsum("e,hed->hd", bk, Whk) + bhk).astype(f32)
    bvH = (np.einsum("e,hed->hd", bv, Whv) + bhv).astype(f32)
    WoM = np.asarray(Wo, f32)

    sel = np.zeros((128, 256), f32)
    for h in range(4):
        sel[32 * h, 64 * h:64 * (h + 1)] = 1.0
    in_maps = []
    for core in range(N_CORES):
        b, g = core // 4, core % 4
        hs = [4 * g + j for j in range(4)]
        Fq_c = np.ascontiguousarray(FqH[:, hs, :].reshape(E, 256))
        Fk_c = np.ascontiguousarray(FkH[:, hs, :].reshape(E, 256))
        Fv_c = np.zeros((E, 260), f32)
        bfv_c = np.zeros((260,), f32)
        for j, h in enumerate(hs):
            Fv_c[:, 65 * j:65 * j + 64] = FvH[:, h, :]
            bfv_c[65 * j:65 * j + 64] = bvH[h]
            bfv_c[65 * j + 64] = 1.0
        bfq_c = np.stack([np.concatenate([bqH[hs[2 * p]], bqH[hs[2 * p + 1]]])
                          for p in range(2)], axis=1)                # [128, 2]
        bfk_c = np.stack([np.concatenate([bkH[hs[2 * p]], bkH[hs[2 * p + 1]]])
                          for p in range(2)], axis=1)
        Wo_c = WoM[256 * g:256 * (g + 1), :]                         # [256, 1024]

        WBqm = Fq_c.reshape(ECH, 128, 256).transpose(1, 0, 2).reshape(128, 2048)
        WBkm = Fk_c.reshape(ECH, 128, 256).transpose(1, 0, 2).reshape(128, 2048)
        WB2m = np.zeros((128, W2_COLS), f32)
        WB2m[:, W2_FV:W2_FV + 2080] = Fv_c.reshape(ECH, 128, 260).transpose(
            1, 0, 2).reshape(128, 2080)
        WB2m[:, W2_WO:W2_WO + 2048] = Wo_c.reshape(2, 128, 1024).transpose(
            1, 0, 2).reshape(128, 2048)
        WB2m[0, W2_BFV:W2_BFV + 260] = bfv_c
        WB2m[0, W2_ONES:W2_ONES + 128] = 1.0

        WFm = np.concatenate([bfq_c, bfk_c], axis=1)                 # [128, 4]

        # keep-mask, transposed: [S(k), S(q)]
        mk2 = (~mask[b].T).astype(f32)

        in_maps.append(dict(
            qT=np.ascontiguousarray(q_matrix[b].T).astype(bf16),
            kT=np.ascontiguousarray(k_matrix[b].T).astype(bf16),
            vT=np.ascontiguousarray(v_matrix[b].T).astype(bf16),
            maskT2=np.ascontiguousarray(mk2).astype(bf16),
            WBk=np.ascontiguousarray(WBkm).astype(bf16),
            WBq=np.ascontiguousarray(WBqm).astype(bf16),
            WB2=WB2m.astype(bf16), WF=WFm, sel=sel,
        ))
    return in_maps


def unshard(results, bo):
    bo = np.asarray(bo, np.float32)
    out = np.empty((B, S, E), np.float32)
    for b in range(B):
        acc = results[4 * b]["out_pT"].astype(np.float32)
        for g in range(1, 4):
            acc = acc + results[4 * b + g]["out_pT"].astype(np.float32)
        out[b] = acc.T + bo
    return out


def kernel(**inputs):
    from concourse.bass_utils import run_bass_kernel_spmd
    nc = build_program()
    in_maps = prep_inputs(**inputs)
    res = run_bass_kernel_spmd(nc, in_maps, list(range(N_CORES)))
    return unshard(res.results, inputs["bo"])
